# revision 1
# baseline (speedup 1.0000x reference)
"""Trainium2 Bass kernel for nn_Attention_16630113370932.

ViT-style attention block:
  x [64, 768, 14, 14] -> 1x1-conv qkv (w_qkv [2304, 768]) -> 12-head attention
  over N=196 tokens (head_dim 64, qk scale 64**-0.25 on both q and k)
  -> 1x1-conv proj (w_proj [768, 768]) -> out [64, 768, 14, 14]

Strategy: pure data-parallel over batch across 8 NeuronCores (8 images per
core, no collectives). All matmuls run in bf16 (fp32 PSUM accumulation);
weights are transposed + bf16-cast on the host. Attention per head computes
scores transposed, ST = K^T Q [m, n], exp on ScalarE; AV pairs the two
heads of a step into disjoint PE column groups (out partitions 0:64/64:128)
so their matmuls overlap in the array. Softmax denominators accumulate into
one [12, 2, N] PSUM tile via one-hot matmuls whose rhs spans both images of
a pair (si-major exp tiles, free (2, N)); normalization is a batched
VectorE reciprocal, a selector matmul that broadcasts each head's recip row
onto its 64 output partitions, and a VectorE multiply. Images are processed
in pairs (392-wide matmul free dim for qkv/proj); the two images' attention
chains interleave at head-pair granularity, and qkv/proj accumulation
half-groups of neighboring pairs are emitted as filler between attention
steps to keep the PE dense (their PSUM evacuations ride DVE only, keeping
ScalarE responsive for the exps).

Scheduling specifics learned from traces:
  - input DMAs are issued in consumption order (pair-0 x, v-cols, q, k,
    rest) so the PE starts ~15us earlier than with monolithic loads;
  - a low-power PE warm-up spinner runs during the input DMAs so HAM
    unthrottles before real work (wide warm matmuls instead trip the P0
    power downclock for the whole kernel: ~2.0GHz, +20% exec);
  - output is written bf16 (cast to f32 on host), per-proj-tile DMAs of
    the last pair alternate the SP/Activation HWDGE queues to shorten the
    drain tail.
"""
import numpy as np
import ml_dtypes

import concourse.bass as bass
import concourse.tile as tile
from concourse import mybir
from concourse.bass_utils import run_bass_kernel_spmd
from concourse.vector_clock import ScopedClock


def _patched_drain_and_barrier(self, tick_clock, wait_clock):
    """TileContext exit-drain carries one sem wait per global-clock proc; the
    walrus build in this image rejects >2 sync waits on a CTRL instruction
    ("Too many sync wait commands"). Keep one wait on the drain and spread the
    rest over single-wait nops on the same (SP) engine — equivalent ordering,
    since all of them precede the all-engine barrier."""
    nc = self.nc
    drain_inst = nc.sync.drain()
    wait_clock.add_sem_waits(
        drain_inst.ins, ScopedClock({None: tick_clock.global_clock})
    )
    si = drain_inst.ins.sync_info
    ow = list(si.on_wait or [])
    if len(ow) > 1:
        si.on_wait = ow[:1]
        for w in ow[1:]:
            nop = nc.sync.nop()
            nop.ins.sync_info = mybir.SyncInfo(on_wait=[w], on_update=[])
    nc.all_engine_barrier()
    assert self.sems is not None
    popped = nc._tile_sem_poison_stack.pop()
    assert popped is self._sem_poison
    nc.clear_and_free_semaphores(list(self.sems.allocated().values()))
    nc.all_engine_barrier()


tile.TileContext._drain_and_barrier = _patched_drain_and_barrier

_MAX_WAITS = 1  # walrus in this image rejects multiple sync waits per TPB instruction


def _split_sync_waits(m: dict) -> dict:
    """Move overflow sem-waits (beyond _MAX_WAITS) onto fresh NoOp
    instructions inserted just before the over-limit instruction on the same
    engine — same-engine program order makes this ordering-equivalent."""
    fresh = [0]
    for f in m.get("functions", []):
        for blk in f.get("blocks", []):
            ins_list = blk.get("instructions", [])
            out = []
            for ins in ins_list:
                si = ins.get("sync_info")
                ow = (si or {}).get("on_wait") or []
                if len(ow) > _MAX_WAITS:
                    eng = ins.get("engine")
                    extra = ow[: len(ow) - _MAX_WAITS]
                    si["on_wait"] = ow[len(ow) - _MAX_WAITS:]
                    for k in range(0, len(extra), _MAX_WAITS):
                        fresh[0] += 1
                        out.append({
                            "debug": ins.get("debug", 0),
                            "engine": eng,
                            "ins": [],
                            "name": f"I-waitsplit-{fresh[0]}",
                            "opcode": "NoOp",
                            "outs": [],
                            "sync_info": {
                                "on_update": [],
                                "on_wait": extra[k:k + _MAX_WAITS],
                            },
                        })
                out.append(ins)
            blk["instructions"] = out
    return m


_orig_to_json_bytes = bass.Bass.to_json_bytes


def _patched_to_json_bytes(self) -> bytes:
    import orjson

    m = orjson.loads(_orig_to_json_bytes(self))
    return orjson.dumps(_split_sync_waits(m))


bass.Bass.to_json_bytes = _patched_to_json_bytes

BF16 = mybir.dt.bfloat16
F32 = mybir.dt.float32

N_CORES = 8
B, CIN, HH, WW = 64, 768, 14, 14
HW = HH * WW            # 196 tokens
NH, DH = 12, 64         # heads, head dim
BPC = B // N_CORES      # 8 images per core
CT = CIN // 128         # 6 contraction tiles
SCALE2 = DH ** -0.5     # 0.125 == (dh**-0.25)**2, folded into w_q on host

_AF = mybir.ActivationFunctionType

# CoreSim rejects the fused exp's read of never-written PSUM rows (68:128 of
# the second score chunk — computed into SBUF but never consumed). Set True
# when building for the simulator to emit per-chunk exps instead.
SIM_SAFE = False


def build_kernel() -> bass.Bass:
    W2 = 2 * HW          # 392: free width of an image pair
    M1 = HW - 128        # 68: second token-chunk size
    nc = bass.Bass()
    xp = nc.declare_dram_parameter("xp", [CT, 128, BPC * HW], BF16, isOutput=False)
    wqkT = nc.declare_dram_parameter("wqkT", [CT, 128, 2304], BF16, isOutput=False)
    wpT = nc.declare_dram_parameter("wpT", [CT, 128, 768], BF16, isOutput=False)
    outp = nc.declare_dram_parameter("out", [CT, 128, BPC * HW], BF16, isOutput=True)

    selp = nc.declare_dram_parameter("sel", [NH, CT * 128], BF16, isOutput=False)
    eyep = nc.declare_dram_parameter("eye", [128, NH * NH], BF16, isOutput=False)

    with tile.TileContext(nc) as tc:
        with (
            tc.tile_pool(name="weights", bufs=1) as wpool,
            tc.tile_pool(name="xin", bufs=1) as xpool,
            tc.tile_pool(name="qk", bufs=2) as qkpool,
            tc.tile_pool(name="vt", bufs=2) as vtpool,
            tc.tile_pool(name="exps", bufs=3) as epool,
            tc.tile_pool(name="rec", bufs=2) as recpool,
            tc.tile_pool(name="ao", bufs=2) as aopool,
            tc.tile_pool(name="osb", bufs=2) as opool,
            tc.tile_pool(name="psA", bufs=3, space="PSUM") as psA,
            tc.tile_pool(name="psST", bufs=2, space="PSUM") as psST,
            tc.tile_pool(name="psAV", bufs=2, space="PSUM") as psAV,
            tc.tile_pool(name="psDEN", bufs=1, space="PSUM") as psDEN,
        ):
            # DMA issue order == consumption order so the PE can start ~15us
            # earlier than a monolithic load: pair-0's x slice and the
            # v-columns of w_qkv feed the first vT groups, then q then k
            # columns (pair-0's qkv j-order is q-tiles before k-tiles), then
            # the remaining pairs' x (needed by the filler qkv of pair 1
            # during pair 0's attention), then w_p (first used ~30us in).
            # All DMAs share the sync-engine HW queue, which drains FIFO
            # across 16 DMA engines at ~310GB/s.
            x_sb = xpool.tile([128, CT, BPC * HW], BF16)
            w_qk = wpool.tile([128, CT, 2304], BF16)
            # (splitting inputs across both HWDGE queues measured +4.7us:
            # the Activation-queue DMA issues delay pair-0's scalar-side
            # work and the single sync queue's ~313GB/s is near enough to
            # the HBM roofline that parallel queues add nothing)
            for t in range(CT):
                nc.sync.dma_start(x_sb[:, t, 0:W2], xp[t][:, 0:W2])
                nc.sync.dma_start(w_qk[:, t, 1536:2304], wqkT[t][:, 1536:2304])
            for t in range(CT):
                nc.sync.dma_start(w_qk[:, t, 0:768], wqkT[t][:, 0:768])
            for t in range(CT):
                nc.sync.dma_start(w_qk[:, t, 768:1536], wqkT[t][:, 768:1536])
            # eyecols[:, h, :] is the 12-col one-hot(h) matrix on every
            # partition: accumulates head h's softmax denominator into row h;
            # sel12[:, j, :].T @ rec12 broadcasts head 2j's recip row onto
            # partitions 0:64 and head 2j+1's onto 64:128.
            eyecols = wpool.tile([128, NH, NH], BF16)
            nc.sync.dma_start(eyecols.rearrange("p h c -> p (h c)"), eyep[:])
            # warm the ScalarE exp table (ACT_TABLE_LOAD ~2.7us) during the
            # input DMAs instead of at the first real exp mid-pipeline
            warm_in = wpool.tile([1, 8], F32)
            warm_out = wpool.tile([1, 8], F32)
            nc.vector.memset(warm_in[:], 0.0)
            nc.scalar.activation(warm_out[:], warm_in[:], _AF.Exp)
            sel12 = wpool.tile([NH, CT, 128], BF16)
            nc.sync.dma_start(sel12.rearrange("h j p -> h (j p)"), selp[:])
            # spin the PE on garbage matmuls while the input DMAs stream, so
            # HAM unthrottles (K=4/8 -> 8/8) before the first real matmul.
            # Low power is critical: wide warm matmuls trip the P0 power
            # downclock (~2.0GHz) for the WHOLE kernel (+38us measured).
            # 8-col output = ~6% of the array active; 64-free keeps the
            # array's busy duty near 100% so the HAM window counts it.
            warm_w = wpool.tile([128, 8], BF16)
            warm_x = wpool.tile([128, 64], BF16)
            nc.vector.memset(warm_w[:], 0.0)
            nc.vector.memset(warm_x[:], 0.0)
            warm_ps = psA.tile([128, 512], F32, tag="psA", name="warmps")
            for _ in range(150):
                nc.tensor.matmul(warm_ps[:8, :64], lhsT=warm_w[:],
                                 rhs=warm_x[:], start=True, stop=True)
            for pr in range(1, BPC // 2):
                for t in range(CT):
                    nc.sync.dma_start(
                        x_sb[:, t, pr * W2:(pr + 1) * W2],
                        xp[t][:, pr * W2:(pr + 1) * W2])
            w_p = wpool.tile([128, CT, 768], BF16)
            for t in range(CT):
                nc.sync.dma_start(w_p[:, t, :], wpT[t])

            from collections import deque
            filler: deque = deque()

            def emit_filler(n):
                for _ in range(min(n, len(filler))):
                    filler.popleft()()

            def qkv_group(qk_sb, x0, j, pos=None, dve_evac=False):
                state = {}

                def fn1():
                    state["ps"] = psA.tile([128, 512], F32, tag="psA",
                                           name=f"psq{x0}_{j}")
                    for t in range(CT // 2):
                        nc.tensor.matmul(
                            state["ps"][:, :W2],
                            lhsT=w_qk[:, t, j * 128:(j + 1) * 128],
                            rhs=x_sb[:, t, x0:x0 + W2],
                            start=(t == 0),
                            stop=False,
                        )

                def fn2():
                    ps = state["ps"]
                    for t in range(CT // 2, CT):
                        nc.tensor.matmul(
                            ps[:, :W2],
                            lhsT=w_qk[:, t, j * 128:(j + 1) * 128],
                            rhs=x_sb[:, t, x0:x0 + W2],
                            start=False,
                            stop=(t == CT - 1),
                        )
                    p_ = j if pos is None else pos
                    if dve_evac or p_ % 2 == 0:
                        # filler-emitted groups evacuate on DVE only: they
                        # run during attention, where ScalarE must stay
                        # responsive for the exps (an exp queued behind a
                        # 580ns copy delays the ST-tile release downstream)
                        nc.vector.tensor_copy(qk_sb[:, j, :], ps[:, :W2])
                    else:
                        nc.scalar.copy(qk_sb[:, j, :], ps[:, :W2])
                return fn1, fn2

            def vt_group(vt, xs, np0, nsz, of):
                def fn():
                    ps = psA.tile([128, 512], F32, tag="psA", name=f"psv{xs}_{np0}_{of}")
                    for t in range(CT):
                        nc.tensor.matmul(
                            ps[:nsz, :384],
                            lhsT=x_sb[:, t, xs + np0:xs + np0 + nsz],
                            rhs=w_qk[:, t, 1536 + of * 384:1536 + (of + 1) * 384],
                            start=(t == 0),
                            stop=(t == CT - 1),
                        )
                    if of == 0:
                        nc.vector.tensor_copy(vt[:nsz, 0:6, :],
                                              ps[:nsz, :384])
                    else:
                        # split vT evacuations across DVE and ScalarE: they
                        # cluster at pair boundaries where DVE is the choke
                        nc.scalar.copy(vt[:nsz, 6:12, :], ps[:nsz, :384])
                return fn

            def proj_group(out_sb, ao_n, x0, j, dma, dve_evac=False):
                state = {}

                def fn1():
                    state["ps"] = psA.tile([128, 512], F32, tag="psA",
                                           name=f"psp{x0}_{j}")
                    for t in range(CT // 2):
                        nc.tensor.matmul(
                            state["ps"][:, :W2],
                            lhsT=w_p[:, t, j * 128:(j + 1) * 128],
                            rhs=ao_n[:, t, :],
                            start=(t == 0),
                            stop=False,
                        )

                def fn2():
                    ps = state["ps"]
                    for t in range(CT // 2, CT):
                        nc.tensor.matmul(
                            ps[:, :W2],
                            lhsT=w_p[:, t, j * 128:(j + 1) * 128],
                            rhs=ao_n[:, t, :],
                            start=False,
                            stop=(t == CT - 1),
                        )
                    if dve_evac or j % 2 != 0:
                        nc.vector.tensor_copy(out_sb[:, j, :], ps[:, :W2])
                    else:
                        nc.scalar.copy(out_sb[:, j, :], ps[:, :W2])
                    if dma == "per_j":
                        # last pair: per-tile DMA so output streams out while
                        # later projections still compute; alternate the two
                        # HWDGE queues (SP/Activation) so the final DMAs'
                        # descriptor processing overlaps (shorter exec tail)
                        dst = outp.rearrange("t p n -> p t n")[:, j, x0:x0 + W2]
                        eng = nc.sync if j % 2 == 0 else nc.scalar
                        eng.dma_start(dst, out_sb[:, j, :])
                    elif dma:
                        dst = outp.rearrange("t p n -> p t n")[:, :, x0:x0 + W2]
                        nc.sync.dma_start(dst, out_sb[:, :, :])
                return fn1, fn2

            qks = {}
            pending_norm = None
            for pr in range(BPC // 2):
                x0 = pr * W2
                pace = {"n": 0}

                def paced_filler():
                    # the last pair has no next-pair qkv to interleave (only
                    # ~12 queue items for 36 points): stretch them across the
                    # whole attention instead of draining in the first steps
                    pace["n"] += 1
                    if pr + 1 == BPC // 2 and pace["n"] % 2 == 0:
                        return
                    if pr == 0 and pace["n"] % 3 == 0:
                        # first pair has only next-pair qkv queued (24 items
                        # for 36 points): pace at 2/3 so it spans all steps
                        return
                    emit_filler(1)
                qk_sb = qks.get(pr)
                ao_un = aopool.tile([128, CT, W2], BF16, tag="aoun")
                ao_n = aopool.tile([128, CT, W2], BF16, tag="aon")

                vts = {}
                for si in range(2):
                    for nch in range(2):
                        vts[(si, nch)] = vtpool.tile(
                            [128, NH, 64], BF16, tag=f"vt{si}_{nch}",
                            name=f"vt{si}{nch}p{pr}")
                # both images' denominators share one PSUM bank: the whole
                # bank's has_written bits clear once (start=True on the very
                # first den matmul of the pair); each region's first write
                # then overwrites, later ones accumulate
                den2 = psDEN.tile([NH, 2, HW], F32, tag="den", name=f"den{pr}")
                den_l = [den2[:, 0, :], den2[:, 1, :]]
                # vT for both images up front (for the first pair this also
                # runs before qkv so its evacuations clear early)
                for si in range(2):
                    for nch, (np0, nsz) in enumerate(((0, 128), (128, M1))):
                        for of in range(2):
                            vt_group(vts[(si, nch)], x0 + si * HW, np0, nsz, of)()
                # previous pair's normalization lands here, AFTER this pair's
                # vT matmuls: its first bcast waits on the reciprocal, and at
                # the boundary that wait would block the in-order PE queue
                # with nothing in front of it
                if pending_norm is not None:
                    pending_norm()
                    pending_norm = None
                if pr == 0:
                    # first pair's q/k projection emitted directly; q-tiles
                    # before k-tiles to match the DMA arrival order (q cols
                    # land before k cols)
                    qks[0] = qkpool.tile([128, 2 * CT, W2], BF16, name="qk0",
                                         tag="qk")
                    for pos, j in enumerate(range(2 * CT)):
                        for f in qkv_group(qks[0], 0, j, pos):
                            f()
                    qk_sb = qks[0]
                if pr + 1 < BPC // 2:
                    # next pair's q/k projection is the attention filler
                    qks[pr + 1] = qkpool.tile(
                        [128, 2 * CT, W2], BF16, name=f"qk{pr + 1}", tag="qk")
                    order = [jj for p in range(CT) for jj in (p, CT + p)]
                    for pos, j in enumerate(order):
                        filler.extend(
                            qkv_group(qks[pr + 1], (pr + 1) * W2, j, pos,
                                      dve_evac=True))

                # ---- attention; the two images' chains interleave at
                # head-pair granularity (independent work hides each
                # chain's exp/evac latency). Heads pair into row groups
                # (ST) / col groups (AV); denominators accumulate into
                # per-image [12, HW] PSUM tiles via one-hot matmuls ----
                rec12 = recpool.tile([NH, W2], BF16, tag="rec12",
                                     name=f"rec{pr}")

                def recip(s0):
                    with nc.allow_low_precision("softmax recip bf16"):
                        nc.vector.reciprocal(rec12[:, s0:s0 + HW],
                                             den_l[s0 // HW][:])
                for hp in range(CT):
                    # both images' exps for a head share one tile [128,2,W2]
                    # (si-major), so each den matmul can take rhs free
                    # (2,196) covering both images: half the den matmul
                    # issues for the same cycles
                    eh = [epool.tile([128, 2, W2], BF16, tag=f"eh{u}",
                                     name=f"e{pr}_{hp}_{u}")
                          for u in range(2)]
                    for si in range(2):
                        s0 = si * HW
                        vt0, vt1 = vts[(si, 0)], vts[(si, 1)]
                        h0, h1 = 2 * hp, 2 * hp + 1
                        # parity-interleaved emission: consecutive matmuls
                        # use disjoint PE row groups (partitions 0:64 vs
                        # 64:128), so they overlap and weight loads pull ahead
                        # (bf16 matmul output to PSUM would let both heads'
                        # scores share one bank, but that's TRN3-only HW)
                        sts = [psST.tile([128, W2], F32, tag="st",
                                         name=f"st{pr}_{si}_{hp}_{u}")
                               for u in range(2)]
                        qs, ks = [], []
                        for h in (h0, h1):
                            po = (h % 2) * 64
                            qs.append(qk_sb[po:po + 64, hp, s0:s0 + HW])
                            ks.append(qk_sb[po:po + 64, CT + hp, s0:s0 + HW])
                        for u in range(2):
                            nc.tensor.matmul(sts[u][:, 0:HW],
                                             lhsT=ks[u][:, 0:128], rhs=qs[u],
                                             start=True, stop=True)
                        for u in range(2):
                            nc.tensor.matmul(sts[u][:M1, HW:W2],
                                             lhsT=ks[u][:, 128:HW], rhs=qs[u],
                                             start=True, stop=True)
                        paced_filler()
                        exps_l = []
                        for u, st in enumerate(sts):
                            exps = eh[u][:, si, :]
                            if SIM_SAFE:
                                nc.scalar.activation(exps[:, 0:HW],
                                                     st[:, 0:HW], _AF.Exp)
                                nc.scalar.activation(exps[:M1, HW:W2],
                                                     st[:M1, HW:W2], _AF.Exp)
                            else:
                                nc.scalar.activation(exps[:, :], st[:, :],
                                                     _AF.Exp)
                            exps_l.append(exps)
                        paced_filler()
                        av = psAV.tile([128, HW], F32, tag="av")
                        # h0/h1 write disjoint PE column groups (out
                        # partitions 0:64 / 64:128), so pairing them lets
                        # consecutive matmuls overlap in the array. The
                        # has_written clear on start=True is per partition
                        # row, so each head's first matmul clears only its
                        # own half and the interleaved accumulation is safe.
                        e0, e1 = exps_l
                        nc.tensor.matmul(av[0:64, :], lhsT=vt0[:, h0, :],
                                         rhs=e0[:, 0:HW],
                                         start=True, stop=False)
                        nc.tensor.matmul(av[64:128, :], lhsT=vt0[:, h1, :],
                                         rhs=e1[:, 0:HW],
                                         start=True, stop=False)
                        nc.tensor.matmul(av[0:64, :], lhsT=vt1[:M1, h0, :],
                                         rhs=e0[:M1, HW:W2],
                                         start=False, stop=True)
                        nc.tensor.matmul(av[64:128, :], lhsT=vt1[:M1, h1, :],
                                         rhs=e1[:M1, HW:W2],
                                         start=False, stop=True)
                        if pr + 1 == BPC // 2:
                            # last pair: per-image dens so image 0's
                            # reciprocal fires an attention-step early — the
                            # tail's norm->proj chain is gated on the
                            # reciprocals with no filler left to hide them
                            for u, h in enumerate((h0, h1)):
                                nc.tensor.matmul(den2[:, si, :],
                                                 lhsT=eyecols[:, h, :],
                                                 rhs=eh[u][:, si, 0:HW],
                                                 start=(h == 0 and si == 0),
                                                 stop=False)
                                nc.tensor.matmul(den2[:, si, :],
                                                 lhsT=eyecols[:M1, h, :],
                                                 rhs=eh[u][:M1, si, HW:W2],
                                                 start=False,
                                                 stop=(h == NH - 1))
                        elif si == 1:
                            # dens for both images at once (rhs free (2,196)
                            # across the si-major exp tile); den2's row
                            # layout [h, (si, q)] is exactly the matmul's
                            # output shape
                            for u, h in enumerate((h0, h1)):
                                nc.tensor.matmul(den2[:, :, :],
                                                 lhsT=eyecols[:, h, :],
                                                 rhs=eh[u][:, :, 0:HW],
                                                 start=(h == 0), stop=False)
                                nc.tensor.matmul(den2[:, :, :],
                                                 lhsT=eyecols[:M1, h, :],
                                                 rhs=eh[u][:M1, :, HW:W2],
                                                 start=False,
                                                 stop=(h == NH - 1))
                        paced_filler()
                        nc.vector.tensor_copy(
                            ao_un[:, hp, s0:s0 + HW], av[:, :])
                        if hp == CT - 1:
                            # reciprocals as soon as the denominators
                            # complete (for the last pair, per image: si=0's
                            # recip hides under si=1's final attention step)
                            if pr + 1 == BPC // 2:
                                recip(s0)
                            elif si == 1:
                                recip(0)
                                recip(HW)

                # leftover queued work (next pair's q/k writes) must be
                # emitted before the next pair's attention reads it
                emit_filler(len(filler))

                def bc_mul(rec12, ao_n, ao_un, pr, j):
                    def fn():
                        bc_ps = psA.tile([128, 512], F32, tag="psA",
                                         name=f"psb{pr}_{j}")
                        nc.tensor.matmul(bc_ps[:, :W2], lhsT=sel12[:, j, :],
                                         rhs=rec12[:], start=True, stop=True)
                        nc.vector.tensor_mul(
                            ao_n[:, j, :], bc_ps[:, :W2], ao_un[:, j, :])
                    return fn

                def make_norm(rec12=rec12, ao_n=ao_n, ao_un=ao_un, pr=pr):
                    def fn():
                        for j in range(CT):
                            bc_mul(rec12, ao_n, ao_un, pr, j)()
                    return fn

                if pr + 1 == BPC // 2:
                    # last pair: per-image norm, image 0 first — its bc
                    # matmuls are gated only on the early recip(0), so the
                    # PE stays busy during recip(1) instead of idling into
                    # a HAM re-throttle
                    for s0_ in (0, HW):
                        for j in range(CT):
                            bc_ps = psA.tile([128, 512], F32, tag="psA",
                                             name=f"psb{pr}_{j}_{s0_}")
                            nc.tensor.matmul(bc_ps[:, 0:HW],
                                             lhsT=sel12[:, j, :],
                                             rhs=rec12[:, s0_:s0_ + HW],
                                             start=True, stop=True)
                            nc.vector.tensor_mul(
                                ao_n[:, j, s0_:s0_ + HW], bc_ps[:, 0:HW],
                                ao_un[:, j, s0_:s0_ + HW])
                else:
                    pending_norm = make_norm()
                emit_filler(len(filler))

                # queue this pair's projection as filler for the next pair's
                # attention; last pair emits directly
                out_sb = opool.tile([128, CT, W2], BF16, tag="osb",
                                    name=f"osb{pr}")
                if pr + 1 == BPC // 2:
                    emit_filler(len(filler))
                last_pair = pr + 1 == BPC // 2
                for j in range(CT):
                    fns = proj_group(out_sb, ao_n, x0, j,
                                     dma="per_j" if last_pair else (j == CT - 1),
                                     dve_evac=not last_pair)
                    if not last_pair:
                        filler.extend(fns)
                    else:
                        for f in fns:
                            f()
            emit_filler(len(filler))
    return nc


_NC_CACHE = None


def _get_nc():
    global _NC_CACHE
    if _NC_CACHE is None:
        _NC_CACHE = build_kernel()
    return _NC_CACHE


def prep_inputs(x: np.ndarray, w_qkv: np.ndarray, w_proj: np.ndarray):
    """Host-side shard + layout prep. Returns in_maps for the 8 cores."""
    bf16 = ml_dtypes.bfloat16
    w = w_qkv.astype(np.float64)
    w = np.concatenate([w[:768] * SCALE2, w[768:]], axis=0)  # fold qk scale into w_q
    wqkT = np.ascontiguousarray(w.T.astype(np.float32).astype(bf16)).reshape(CT, 128, 2304)
    wpT = np.ascontiguousarray(w_proj.T.astype(bf16)).reshape(CT, 128, 768)
    sel = np.zeros((NH, CT, 128), dtype=bf16)
    for j in range(CT):
        sel[2 * j, j, 0:64] = 1
        sel[2 * j + 1, j, 64:128] = 1
    sel = sel.reshape(NH, CT * 128)
    eye = np.broadcast_to(np.eye(NH, dtype=bf16), (128, NH, NH))
    eye = np.ascontiguousarray(eye).reshape(128, NH * NH)
    xr = x.reshape(B, CIN, HW)
    in_maps = []
    for c in range(N_CORES):
        xs = xr[c * BPC:(c + 1) * BPC]                      # [8, 768, 196]
        xs = xs.reshape(BPC, CT, 128, HW).transpose(1, 2, 0, 3)  # [6, 128, 8, 196]
        xs = np.ascontiguousarray(xs).astype(bf16).reshape(CT, 128, BPC * HW)
        in_maps.append({"xp": xs, "wqkT": wqkT, "wpT": wpT, "sel": sel, "eye": eye})
    return in_maps


def run(x, w_qkv, w_proj, trace=False, trace_kwargs=None):
    nc = _get_nc()
    in_maps = prep_inputs(x, w_qkv, w_proj)
    res = run_bass_kernel_spmd(
        nc, in_maps, core_ids=list(range(N_CORES)), trace=trace,
        **(trace_kwargs or {}),
    )
    outs = []
    for c in range(N_CORES):
        o = res.results[c]["out"].astype(np.float32)          # bf16 -> f32
        o = o.reshape(CT, 128, BPC, HW)
        o = o.transpose(2, 0, 1, 3).reshape(BPC, CIN, HH, WW)
        outs.append(o)
    full = np.concatenate(outs, axis=0).astype(np.float32)
    return full, res


def kernel(x: np.ndarray, w_qkv: np.ndarray, w_proj: np.ndarray) -> np.ndarray:
    out, _ = run(x, w_qkv, w_proj, trace=False)
    return out


if __name__ == "__main__":
    rng = np.random.default_rng(0)
    x = rng.standard_normal((B, CIN, HH, WW), dtype=np.float32)
    w_qkv = (rng.standard_normal((2304, 768), dtype=np.float32) * 0.02).astype(np.float32)
    w_proj = (rng.standard_normal((768, 768), dtype=np.float32) * 0.02).astype(np.float32)
    out = kernel(x, w_qkv, w_proj)
    print("out", out.shape, out.dtype)

